# revision 35
# baseline (speedup 1.0000x reference)
"""Distributed multi-head attention forward for 8 TRN2 NeuronCores.

Problem: B=2, N=2048, D=768, 12 heads x 64 head-dim, f32.
  qkv = x @ w_qkv + b_qkv ; per-head softmax(q k^T / 8) v ; out proj.

Sharding: core = 4*b + g (b = batch element, g = query-chunk of 512 rows).
No collectives: every core receives the FULL x^T of its batch (bf16,
host-transposed, token-rotated so its own 512 query rows sit first) and
replicates the K^T / V projections for all 2048 keys locally — on this part
the 55us+ fixed cost of a 4-core ring AllGather loses to ~60us of extra
bf16 matmuls that pipeline perfectly.

Schedule (single PE stream, everything else slotted around it):
  Q proj -> K proj ct 0-1 -> attention j=0 with the 16 V-projection steps
  interleaved chunk-by-chunk -> attention j=1..4 with K proj ct 2-5 spread
  as PE filler -> attention j=5 -> output projection.  Each head pair's
  finalize (den -> ones-broadcast matmul -> reciprocal_approx_fast ->
  multiply) is deferred into the next pair's chunk loop.  PSUM: S tiles
  3-deep (6 banks) + one PV accumulator pair (2 banks).

Layouts: all activations transposed ([cols, tokens]) except V (natural),
everything bf16 on the wire and in SBUF; psum accumulation f32.  V carries
a per-head ones column so P@V also yields the softmax denominator; the V
bias is folded into the output bias on the host (sum(P)=1).
"""

import numpy as np

import concourse.bass as bass
import concourse.tile as tile
from concourse import bacc, mybir
from concourse.bass import ts, ds
from concourse.bass_utils import run_bass_kernel_spmd

FP = mybir.dt.float32
FR = mybir.dt.float32r
BF = mybir.dt.bfloat16

P = 128
T = 512            # query rows per core
D = 768            # model dim
H = 12             # heads
DH = 64            # head dim
VA = H * (DH + 1)  # 780: v columns + per-head ones column
KEYS = 2048
DC = D // P        # 6 chunks of the contraction dim
NKC = KEYS // P    # 16 key chunks of 128
NKT = KEYS // T    # 4 key chunks of 512
SCALE = DH ** -0.5


def build_nc():
    nc = bacc.Bacc(
        "TRN2",
        target_bir_lowering=False,
        debug=False,
        enable_asserts=False,
        num_devices=8,
    )
    import os
    dbg = {}
    for name, shape in (
        ("dQT", [P, DC, T]), ("dKT", [P, DC, KEYS]),
        ("dV", [P, NKC, VA]), ("dOT", [P, DC, T]),
    ):
        if name[1:] in os.environ.get("KDBG", "").split(","):
            dbg[name[1:]] = nc.dram_tensor(name, shape, BF, kind="ExternalOutput").ap()

    xT = nc.dram_tensor("xT", [D, KEYS], BF, kind="ExternalInput").ap()
    wq = nc.dram_tensor("wq", [DC, P, DC, P], BF, kind="ExternalInput").ap()
    wk = nc.dram_tensor("wk", [DC, P, DC, P], BF, kind="ExternalInput").ap()
    wv = nc.dram_tensor("wv", [D, VA], BF, kind="ExternalInput").ap()
    bq = nc.dram_tensor("bq", [D], FP, kind="ExternalInput").ap()
    bk = nc.dram_tensor("bk", [D], FP, kind="ExternalInput").ap()
    wo = nc.dram_tensor("wo", [D, D], BF, kind="ExternalInput").ap()
    bo = nc.dram_tensor("bo", [D], FP, kind="ExternalInput").ap()
    out = nc.dram_tensor("out", [T, D], FP, kind="ExternalOutput").ap()

    with tile.TileContext(nc) as tc:
        _build_body(tc, xT, wq, wk, wv, bq, bk, wo, bo, out, dbg)
    nc.compile()
    return nc


def _build_body(tc, xT_d, wq, wk, wv, bq, bk, wo, bo, out, dbg=None):
    nc = tc.nc
    Add = mybir.AluOpType.add
    Mult = mybir.AluOpType.mult
    Exp = mybir.ActivationFunctionType.Exp

    big = tc.alloc_tile_pool(name="big", bufs=1)
    stream = tc.alloc_tile_pool(name="stream", bufs=2)
    singles = tc.alloc_tile_pool(name="singles", bufs=1)
    psum = tc.alloc_tile_pool(name="psum", bufs=2, space="PSUM")

    # b2: [128, 1024] f32 = 2 psum banks; bufs=3 -> 6 banks.
    def b2(name):
        return psum.tile([P, 2 * T], FP, tag="b2", bufs=3, name=name)

    # pv: attention accumulator, 2 banks, single-buffered.
    def bpv(name):
        return psum.tile([P, 2 * T], FP, tag="pv", bufs=1, name=name)

    # ---- persistent SBUF tensors ----
    xT = big.tile([P, DC, KEYS], BF)     # x^T, all tokens (rotated)
    QT = big.tile([P, DC, T], BF)        # Q^T for own 512 rows (biased)
    KT = big.tile([P, DC, KEYS], BF)     # K^T all keys (biased)
    V = big.tile([P, NKC, VA], BF)       # V all keys (+ones cols)
    OT = big.tile([P, DC, T], BF)        # attention output, transposed
    wv_sb = big.tile([P, DC, VA], BF)
    wo_sb = big.tile([P, DC, D], BF)

    # ---- constants ----
    ones_bf = singles.tile([1, DH], BF)
    nc.vector.memset(ones_bf, 1.0)
    bq_sb = singles.tile([P, DC], FP)
    bk_sb = singles.tile([P, DC], FP)
    bo_bc = singles.tile([P, D], FP)

    # ---- input DMAs: prioritize what phase 1 (Q proj) needs ----
    wq_sb = big.tile([P, DC, DC, P], BF)   # [p, ct, o, c]
    wk_sb = big.tile([P, DC, DC, P], BF)
    for dc in range(DC):
        nc.sync.dma_start(xT[:, dc, 0:T], xT_d[ts(dc, P), 0:T])
    for ct in range(DC):
        nc.sync.dma_start(wq_sb[:, ct, :, :], wq[ct])
    nc.sync.dma_start(bq_sb, bq.rearrange("(o p) -> p o", p=P))
    nc.sync.dma_start(bk_sb, bk.rearrange("(o p) -> p o", p=P))
    for ct in range(DC):
        nc.sync.dma_start(wk_sb[:, ct, :, :], wk[ct])
    for dc in range(DC):
        nc.sync.dma_start(xT[:, dc, T:KEYS], xT_d[ts(dc, P), T:KEYS])
    for dc in range(DC):
        nc.sync.dma_start(wv_sb[:, dc, :], wv[ts(dc, P), :])
    for dc in range(DC):
        nc.sync.dma_start(wo_sb[:, dc, :], wo[ts(dc, P), :])
    nc.gpsimd.dma_start(
        out=bo_bc, in_=bass.AP(tensor=bo.tensor, offset=bo.offset, ap=[[0, P], *bo.ap])
    )

    # ---- phase 1: Q^T projection (own 512 rows) ----
    for ct in range(DC):
        pq = b2("pq")
        for dc in range(DC):
            nc.tensor.matmul(
                pq[:, :T], wq_sb[:, ct, dc, :], xT[:, dc, 0:T],
                start=(dc == 0), stop=(dc == DC - 1),
            )
        nc.scalar.add(QT[:, ct, :], pq[:, :T], bq_sb[:, ct : ct + 1])

    # ---- phase 2: K^T projection; ct 0-1 upfront, ct 2-5 interleaved into
    # the attention loop as PE filler work.
    def k_group(ct, kc):
        pk = b2("pk")
        for dc in range(DC):
            nc.tensor.matmul(
                pk[:, :T], wk_sb[:, ct, dc, :], xT[:, dc, ts(kc, T)],
                start=(dc == 0), stop=(dc == DC - 1),
            )
        nc.scalar.add(KT[:, ct, ts(kc, T)], pk[:, :T], bk_sb[:, ct : ct + 1])

    for ct in range(2):
        for kc in range(NKT):
            k_group(ct, kc)

    # ---- phase 3+4: V projection (all keys) interleaved with attention j=0
    # V tile tt covers key chunk c=tt (128 tokens); attention consumes chunks
    # in the same order, so j=0 can run inside the V loop.
    def v_step(tt):
        pv = b2("pvproj")
        for dc in range(DC):
            for lo, sz in ((0, T), (T, VA - T)):
                nc.tensor.matmul(
                    pv[:, ds(lo, sz)],
                    xT[:, dc, ts(tt, P)],
                    wv_sb[:, dc, ds(lo, sz)],
                    start=(dc == 0), stop=(dc == DC - 1),
                )
        nc.vector.tensor_copy(out=V[:, tt, :], in_=pv[:, :VA])
        ones_ap = V[:, tt, :].rearrange("p (h d1) -> p h d1", d1=DH + 1)[:, :, DH]
        nc.vector.memset(ones_ap, 1.0)

    def attn_j(j, interleave_v=False, fill_k=(), fin_prev=None):
        """Attention for head pair (2j, 2j+1) over all 16 key chunks.
        Returns a finalize closure (run it one j later to pipeline).
        If interleave_v, the V-projection steps are interleaved; fill_k
        closures are spread across the chunk loop as PE filler work."""
        fill_k = list(fill_k)
        if fin_prev is not None:
            fin_prev()
        pv_acc = None  # allocated lazily at the first PV accumulation
        ps_tiles = {}

        def s_step(c):
            ps = b2(f"ps{j}_{c}")
            ps_tiles[c] = ps
            for hl, off in ((0, 0), (1, DH)):
                nc.tensor.matmul(
                    ps[:, ds(hl * T, T)],
                    KT[ds(off, DH), j, ts(c, P)],
                    QT[ds(off, DH), j, :],
                    start=True, stop=True,
                )

        if interleave_v:
            v_step(0)
        s_step(0)
        for c in range(NKC):
            es = stream.tile([P, 2 * T], BF, tag="expS", bufs=4, name="es")
            nc.scalar.activation(es, ps_tiles[c][:, :], Exp, scale=SCALE)
            if c + 1 < NKC:
                s_step(c + 1)
                if interleave_v:
                    v_step(c + 1)
            if fill_k and c % 4 == 1:
                fill_k.pop(0)()
            if pv_acc is None:
                pv_acc = bpv(f"pv{j}")  # h0: [:65, :512], h1: [:65, 512:]
            for hl in (0, 1):
                nc.tensor.matmul(
                    pv_acc[: DH + 1, ds(hl * T, T)],
                    V[:, c, ds((2 * j + hl) * (DH + 1), DH + 1)],
                    es[:, ds(hl * T, T)],
                    start=(c == 0), stop=(c == NKC - 1),
                )

        def finalize():
            den_bf = stream.tile([1, 2 * T], BF, tag="den", bufs=2, name="den_bf")
            nc.scalar.copy(den_bf, pv_acc[DH : DH + 1, :])
            bc = b2(f"bc{j}")
            for hl in (0, 1):
                nc.tensor.matmul(
                    bc[:DH, ds(hl * T, T)], ones_bf, den_bf[:, ds(hl * T, T)],
                    start=True, stop=True,
                )
            bc_sb = stream.tile([DH, 2 * T], FP, tag="bcs", bufs=2, name="bc_sb")
            nc.scalar.copy(bc_sb, bc[:DH, :])
            recip = stream.tile([DH, 2 * T], FP, tag="recip", bufs=2, name="recip")
            nc.vector.reciprocal_approx_fast(out=recip, in_=bc_sb)
            for hl in (0, 1):
                nc.vector.tensor_tensor(
                    out=OT[ds(hl * DH, DH), j, :],
                    in0=pv_acc[:DH, ds(hl * T, T)],
                    in1=recip[:, ds(hl * T, T)], op=Mult,
                )

        return finalize

    fin = None
    for j in range(DC):
        fill_k = ()
        if 1 <= j <= 4:
            ct = j + 1
            fill_k = tuple(
                (lambda ct=ct, kc=kc: k_group(ct, kc)) for kc in range(NKT)
            )
        fin = attn_j(j, interleave_v=(j == 0), fill_k=fill_k, fin_prev=fin)
    fin()

    # ---- phase 6: output projection ----
    for tt in range(T // P):
        po = b2("po")
        for dc in range(DC):
            for lo, sz in ((0, T), (T, D - T)):
                nc.tensor.matmul(
                    po[:, ds(lo, sz)],
                    OT[:, dc, ts(tt, P)],
                    wo_sb[:, dc, ds(lo, sz)],
                    start=(dc == 0), stop=(dc == DC - 1),
                )
        o_stage = stream.tile([P, D], FP, tag="ost", bufs=2, name="o_stage")
        nc.vector.tensor_tensor(out=o_stage, in0=po[:, :D], in1=bo_bc, op=Add)
        nc.sync.dma_start(out[ts(tt, P), :], o_stage)

    if dbg:
        tiles = {"QT": QT, "KT": KT, "V": V, "OT": OT}
        for name, dap in dbg.items():
            nc.sync.dma_start(dap, tiles[name])

    for pool in (psum, singles, stream, big):
        pool.release()


_CACHE = {}


def _get_nc():
    if "nc" not in _CACHE:
        _CACHE["nc"] = build_nc()
    return _CACHE["nc"]


def _prep_inputs(x, w_qkv, b_qkv, w_out, b_out):
    import ml_dtypes

    bf16 = ml_dtypes.bfloat16
    x = np.asarray(x, np.float32)
    w_qkv = np.asarray(w_qkv, np.float32)
    b_qkv = np.asarray(b_qkv, np.float32)
    w_out = np.asarray(w_out, np.float32)
    b_out = np.asarray(b_out, np.float32)

    wq_n = w_qkv[:, 0:768]
    wk_n = w_qkv[:, 768:1536]
    wv_raw = w_qkv[:, 1536:2304]
    bq = np.ascontiguousarray(b_qkv[0:768])
    bk = np.ascontiguousarray(b_qkv[768:1536])
    bv_raw = b_qkv[1536:2304]

    # [ct, p, o, c] layout so the per-ct stationary DMA is contiguous
    def w_re(w):
        return np.ascontiguousarray(
            w.reshape(DC, P, DC, P).transpose(2, 1, 0, 3).astype(bf16)
        )

    wq_r = w_re(wq_n)
    wk_r = w_re(wk_n)

    wv = np.zeros((D, VA), np.float32)
    for h in range(H):
        wv[:, h * 65 : h * 65 + 64] = wv_raw[:, h * 64 : (h + 1) * 64]
    wv = wv.astype(bf16)
    # V bias folds into the output bias: softmax rows sum to 1.
    bo_eff = (b_out + bv_raw @ w_out).astype(np.float32)
    wo = np.ascontiguousarray(w_out.astype(bf16))

    in_maps = []
    for b in range(2):
        xb = x[b]
        for g in range(4):
            xrot = np.roll(xb, -g * T, axis=0)
            xTb = np.ascontiguousarray(xrot.T.astype(bf16))
            in_maps.append(
                dict(
                    xT=xTb, wq=wq_r, wk=wk_r, wv=wv, bq=bq, bk=bk,
                    wo=wo, bo=bo_eff,
                )
            )
    return in_maps


def run_on_hw(x, w_qkv, b_qkv, w_out, b_out, **kwargs):
    in_maps = _prep_inputs(x, w_qkv, b_qkv, w_out, b_out)
    res = run_bass_kernel_spmd(_get_nc(), in_maps, core_ids=list(range(8)), **kwargs)
    full = np.empty((2, 2048, D), np.float32)
    for b in range(2):
        for g in range(4):
            full[b, g * T : (g + 1) * T] = res.results[b * 4 + g]["out"]
    return full, res


def kernel(x, w_qkv, b_qkv, w_out, b_out):
    full, _ = run_on_hw(x, w_qkv, b_qkv, w_out, b_out)
    return full


# revision 37
# speedup vs baseline: 1.0956x; 1.0956x over previous
"""Distributed multi-head attention forward for 8 TRN2 NeuronCores.

Problem: B=2, N=2048, D=768, 12 heads x 64 head-dim, f32.
  qkv = x @ w_qkv + b_qkv ; per-head softmax(q k^T / 8) v ; out proj.

Sharding: core = 4*b + g (b = batch element, g = query-chunk of 512 rows).
No collectives: every core receives the FULL x^T of its batch (bf16,
host-transposed, token-rotated so its own 512 query rows sit first) and
replicates the K^T / V projections for all 2048 keys locally — on this part
the 55us+ fixed cost of a 4-core ring AllGather loses to ~60us of extra
bf16 matmuls that pipeline perfectly.

Schedule (single PE stream, everything else slotted around it):
  Q proj -> K proj ct 0-1 -> attention j=0 with the 16 V-projection steps
  interleaved chunk-by-chunk -> attention j=1..4 with K proj ct 2-5 spread
  as PE filler -> attention j=5 -> output projection.  Each head pair's
  finalize (den -> ones-broadcast matmul -> reciprocal_approx_fast ->
  multiply) is deferred into the next pair's chunk loop.  PSUM: S tiles
  3-deep (6 banks) + one PV accumulator pair (2 banks).

Layouts: all activations transposed ([cols, tokens]) except V (natural),
everything bf16 on the wire and in SBUF; psum accumulation f32.  V carries
a per-head ones column so P@V also yields the softmax denominator; the V
bias is folded into the output bias on the host (sum(P)=1).
"""

import numpy as np

import concourse.bass as bass
import concourse.tile as tile
from concourse import bacc, mybir
from concourse.bass import ts, ds
from concourse.bass_utils import run_bass_kernel_spmd

FP = mybir.dt.float32
FR = mybir.dt.float32r
BF = mybir.dt.bfloat16

P = 128
T = 512            # query rows per core
D = 768            # model dim
H = 12             # heads
DH = 64            # head dim
VA = H * (DH + 1)  # 780: v columns + per-head ones column
KEYS = 2048
DC = D // P        # 6 chunks of the contraction dim
NKC = KEYS // P    # 16 key chunks of 128
NKT = KEYS // T    # 4 key chunks of 512
SCALE = DH ** -0.5


def build_nc():
    nc = bacc.Bacc(
        "TRN2",
        target_bir_lowering=False,
        debug=False,
        enable_asserts=False,
        num_devices=8,
    )
    import os
    dbg = {}
    for name, shape in (
        ("dQT", [P, DC, T]), ("dKT", [P, DC, KEYS]),
        ("dV", [P, NKC, VA]), ("dOT", [P, DC, T]),
    ):
        if name[1:] in os.environ.get("KDBG", "").split(","):
            dbg[name[1:]] = nc.dram_tensor(name, shape, BF, kind="ExternalOutput").ap()

    xT = nc.dram_tensor("xT", [D, KEYS], BF, kind="ExternalInput").ap()
    wq = nc.dram_tensor("wq", [DC, P, DC, P], BF, kind="ExternalInput").ap()
    wk = nc.dram_tensor("wk", [DC, P, DC, P], BF, kind="ExternalInput").ap()
    wv = nc.dram_tensor("wv", [D, VA], BF, kind="ExternalInput").ap()
    bq = nc.dram_tensor("bq", [D], FP, kind="ExternalInput").ap()
    bk = nc.dram_tensor("bk", [D], FP, kind="ExternalInput").ap()
    wo = nc.dram_tensor("wo", [D, D], BF, kind="ExternalInput").ap()
    bo = nc.dram_tensor("bo", [D], FP, kind="ExternalInput").ap()
    out = nc.dram_tensor("out", [T, D], FP, kind="ExternalOutput").ap()

    with tile.TileContext(nc) as tc:
        _build_body(tc, xT, wq, wk, wv, bq, bk, wo, bo, out, dbg)
    nc.compile()
    return nc


def _build_body(tc, xT_d, wq, wk, wv, bq, bk, wo, bo, out, dbg=None):
    nc = tc.nc
    Add = mybir.AluOpType.add
    Mult = mybir.AluOpType.mult
    Exp = mybir.ActivationFunctionType.Exp

    big = tc.alloc_tile_pool(name="big", bufs=1)
    stream = tc.alloc_tile_pool(name="stream", bufs=2)
    singles = tc.alloc_tile_pool(name="singles", bufs=1)
    psum = tc.alloc_tile_pool(name="psum", bufs=2, space="PSUM")

    # b2: [128, 1024] f32 = 2 psum banks; bufs=3 -> 6 banks.
    def b2(name):
        return psum.tile([P, 2 * T], FP, tag="b2", bufs=3, name=name)

    # pv: attention accumulator, 2 banks, single-buffered.
    def bpv(name):
        return psum.tile([P, 2 * T], FP, tag="pv", bufs=1, name=name)

    # ---- persistent SBUF tensors ----
    xT = big.tile([P, DC, KEYS], BF)     # x^T, all tokens (rotated)
    QT = big.tile([P, DC, T], BF)        # Q^T for own 512 rows (biased)
    KT = big.tile([P, DC, KEYS], BF)     # K^T all keys (biased)
    V = big.tile([P, NKC, VA], BF)       # V all keys (+ones cols)
    OT = big.tile([P, DC, T], BF)        # attention output, transposed
    wv_sb = big.tile([P, DC, VA], BF)
    wo_sb = big.tile([P, DC, D], BF)

    # ---- constants ----
    ones_bf = singles.tile([1, DH], BF)
    nc.vector.memset(ones_bf, 1.0)
    bq_sb = singles.tile([P, DC], FP)
    bk_sb = singles.tile([P, DC], FP)
    bo_bc = singles.tile([P, D], FP)

    # ---- input DMAs: prioritize what phase 1 (Q proj) needs ----
    wq_sb = big.tile([P, DC, DC, P], BF)   # [p, ct, o, c]
    wk_sb = big.tile([P, DC, DC, P], BF)
    for dc in range(DC):
        nc.sync.dma_start(xT[:, dc, 0:T], xT_d[ts(dc, P), 0:T])
    for ct in range(DC):
        nc.sync.dma_start(wq_sb[:, ct, :, :], wq[ct])
    nc.sync.dma_start(bq_sb, bq.rearrange("(o p) -> p o", p=P))
    nc.sync.dma_start(bk_sb, bk.rearrange("(o p) -> p o", p=P))
    for ct in range(DC):
        nc.sync.dma_start(wk_sb[:, ct, :, :], wk[ct])
    for dc in range(DC):
        nc.sync.dma_start(xT[:, dc, T:KEYS], xT_d[ts(dc, P), T:KEYS])
    for dc in range(DC):
        nc.sync.dma_start(wv_sb[:, dc, :], wv[ts(dc, P), :])
    for dc in range(DC):
        nc.sync.dma_start(wo_sb[:, dc, :], wo[ts(dc, P), :])
    nc.gpsimd.dma_start(
        out=bo_bc, in_=bass.AP(tensor=bo.tensor, offset=bo.offset, ap=[[0, P], *bo.ap])
    )

    # ---- phase 1: Q^T projection (own 512 rows) ----
    for ct in range(DC):
        pq = b2("pq")
        for dc in range(DC):
            nc.tensor.matmul(
                pq[:, :T], wq_sb[:, ct, dc, :], xT[:, dc, 0:T],
                start=(dc == 0), stop=(dc == DC - 1),
            )
        nc.scalar.add(QT[:, ct, :], pq[:, :T], bq_sb[:, ct : ct + 1])

    # ---- phase 2: K^T projection; ct 0-1 upfront, ct 2-5 interleaved into
    # the attention loop as PE filler work.
    def k_group(ct, kc):
        pk = b2("pk")
        for dc in range(DC):
            nc.tensor.matmul(
                pk[:, :T], wk_sb[:, ct, dc, :], xT[:, dc, ts(kc, T)],
                start=(dc == 0), stop=(dc == DC - 1),
            )
        nc.scalar.add(KT[:, ct, ts(kc, T)], pk[:, :T], bk_sb[:, ct : ct + 1])

    for ct in range(2):
        for kc in range(NKT):
            k_group(ct, kc)

    # ---- phase 3+4: V projection (all keys) interleaved with attention j=0
    # V tile tt covers key chunk c=tt (128 tokens); attention consumes chunks
    # in the same order, so j=0 can run inside the V loop.
    def v_step(tt):
        pv = b2("pvproj")
        for dc in range(DC):
            for lo, sz in ((0, T), (T, VA - T)):
                nc.tensor.matmul(
                    pv[:, ds(lo, sz)],
                    xT[:, dc, ts(tt, P)],
                    wv_sb[:, dc, ds(lo, sz)],
                    start=(dc == 0), stop=(dc == DC - 1),
                )
        nc.vector.tensor_copy(out=V[:, tt, :], in_=pv[:, :VA])
        ones_ap = V[:, tt, :].rearrange("p (h d1) -> p h d1", d1=DH + 1)[:, :, DH]
        nc.vector.memset(ones_ap, 1.0)

    def attn_j(j, interleave_v=False, fill_k=(), fin_prev=None):
        """Attention for head pair (2j, 2j+1) over all 16 key chunks.
        Returns a finalize closure (run it one j later to pipeline).
        If interleave_v, the V-projection steps are interleaved; fill_k
        closures are spread across the chunk loop as PE filler work."""
        fill_k = list(fill_k)
        pv_acc = None  # allocated lazily at the first PV accumulation
        ps_tiles = {}

        def s_step(c):
            ps = b2(f"ps{j}_{c}")
            ps_tiles[c] = ps
            for hl, off in ((0, 0), (1, DH)):
                nc.tensor.matmul(
                    ps[:, ds(hl * T, T)],
                    KT[ds(off, DH), j, ts(c, P)],
                    QT[ds(off, DH), j, :],
                    start=True, stop=True,
                )

        for c0 in range(2):
            if interleave_v:
                v_step(c0)
            s_step(c0)
        for c in range(NKC):
            es = stream.tile([P, 2 * T], BF, tag="expS", bufs=4, name="es")
            nc.scalar.activation(es, ps_tiles[c][:, :], Exp, scale=SCALE)
            if c == 0 and fin_prev is not None:
                fin_prev()
            if c + 2 < NKC:
                s_step(c + 2)
                if interleave_v:
                    v_step(c + 2)
            if fill_k and c % 4 == 1:
                fill_k.pop(0)()
            if pv_acc is None:
                pv_acc = bpv(f"pv{j}")  # h0: [:65, :512], h1: [:65, 512:]
            for hl in (0, 1):
                nc.tensor.matmul(
                    pv_acc[: DH + 1, ds(hl * T, T)],
                    V[:, c, ds((2 * j + hl) * (DH + 1), DH + 1)],
                    es[:, ds(hl * T, T)],
                    start=(c == 0), stop=(c == NKC - 1),
                )

        def finalize():
            den_bf = stream.tile([1, 2 * T], BF, tag="den", bufs=2, name="den_bf")
            nc.vector.tensor_copy(out=den_bf, in_=pv_acc[DH : DH + 1, :])
            bc = b2(f"bc{j}")
            for hl in (0, 1):
                nc.tensor.matmul(
                    bc[:DH, ds(hl * T, T)], ones_bf, den_bf[:, ds(hl * T, T)],
                    start=True, stop=True,
                )
            bc_sb = stream.tile([DH, 2 * T], FP, tag="bcs", bufs=2, name="bc_sb")
            nc.vector.tensor_copy(out=bc_sb, in_=bc[:DH, :])
            recip = stream.tile([DH, 2 * T], FP, tag="recip", bufs=2, name="recip")
            nc.vector.reciprocal_approx_fast(out=recip, in_=bc_sb)
            for hl in (0, 1):
                nc.vector.tensor_tensor(
                    out=OT[ds(hl * DH, DH), j, :],
                    in0=pv_acc[:DH, ds(hl * T, T)],
                    in1=recip[:, ds(hl * T, T)], op=Mult,
                )

        return finalize

    fin = None
    for j in range(DC):
        fill_k = ()
        if 1 <= j <= 4:
            ct = j + 1
            fill_k = tuple(
                (lambda ct=ct, kc=kc: k_group(ct, kc)) for kc in range(NKT)
            )
        fin = attn_j(j, interleave_v=(j == 0), fill_k=fill_k, fin_prev=fin)
    fin()

    # ---- phase 6: output projection ----
    for tt in range(T // P):
        po = b2("po")
        for dc in range(DC):
            for lo, sz in ((0, T), (T, D - T)):
                nc.tensor.matmul(
                    po[:, ds(lo, sz)],
                    OT[:, dc, ts(tt, P)],
                    wo_sb[:, dc, ds(lo, sz)],
                    start=(dc == 0), stop=(dc == DC - 1),
                )
        o_stage = stream.tile([P, D], FP, tag="ost", bufs=2, name="o_stage")
        nc.vector.tensor_tensor(out=o_stage, in0=po[:, :D], in1=bo_bc, op=Add)
        nc.sync.dma_start(out[ts(tt, P), :], o_stage)

    if dbg:
        tiles = {"QT": QT, "KT": KT, "V": V, "OT": OT}
        for name, dap in dbg.items():
            nc.sync.dma_start(dap, tiles[name])

    for pool in (psum, singles, stream, big):
        pool.release()


_CACHE = {}


def _get_nc():
    if "nc" not in _CACHE:
        _CACHE["nc"] = build_nc()
    return _CACHE["nc"]


def _prep_inputs(x, w_qkv, b_qkv, w_out, b_out):
    import ml_dtypes

    bf16 = ml_dtypes.bfloat16
    x = np.asarray(x, np.float32)
    w_qkv = np.asarray(w_qkv, np.float32)
    b_qkv = np.asarray(b_qkv, np.float32)
    w_out = np.asarray(w_out, np.float32)
    b_out = np.asarray(b_out, np.float32)

    wq_n = w_qkv[:, 0:768]
    wk_n = w_qkv[:, 768:1536]
    wv_raw = w_qkv[:, 1536:2304]
    bq = np.ascontiguousarray(b_qkv[0:768])
    bk = np.ascontiguousarray(b_qkv[768:1536])
    bv_raw = b_qkv[1536:2304]

    # [ct, p, o, c] layout so the per-ct stationary DMA is contiguous
    def w_re(w):
        return np.ascontiguousarray(
            w.reshape(DC, P, DC, P).transpose(2, 1, 0, 3).astype(bf16)
        )

    wq_r = w_re(wq_n)
    wk_r = w_re(wk_n)

    wv = np.zeros((D, VA), np.float32)
    for h in range(H):
        wv[:, h * 65 : h * 65 + 64] = wv_raw[:, h * 64 : (h + 1) * 64]
    wv = wv.astype(bf16)
    # V bias folds into the output bias: softmax rows sum to 1.
    bo_eff = (b_out + bv_raw @ w_out).astype(np.float32)
    wo = np.ascontiguousarray(w_out.astype(bf16))

    in_maps = []
    for b in range(2):
        xb = x[b]
        for g in range(4):
            xrot = np.roll(xb, -g * T, axis=0)
            xTb = np.ascontiguousarray(xrot.T.astype(bf16))
            in_maps.append(
                dict(
                    xT=xTb, wq=wq_r, wk=wk_r, wv=wv, bq=bq, bk=bk,
                    wo=wo, bo=bo_eff,
                )
            )
    return in_maps


def run_on_hw(x, w_qkv, b_qkv, w_out, b_out, **kwargs):
    in_maps = _prep_inputs(x, w_qkv, b_qkv, w_out, b_out)
    res = run_bass_kernel_spmd(_get_nc(), in_maps, core_ids=list(range(8)), **kwargs)
    full = np.empty((2, 2048, D), np.float32)
    for b in range(2):
        for g in range(4):
            full[b, g * T : (g + 1) * T] = res.results[b * 4 + g]["out"]
    return full, res


def kernel(x, w_qkv, b_qkv, w_out, b_out):
    full, _ = run_on_hw(x, w_qkv, b_qkv, w_out, b_out)
    return full


# revision 39
# speedup vs baseline: 1.1011x; 1.0050x over previous
"""Distributed multi-head attention forward for 8 TRN2 NeuronCores.

Problem: B=2, N=2048, D=768, 12 heads x 64 head-dim, f32.
  qkv = x @ w_qkv + b_qkv ; per-head softmax(q k^T / 8) v ; out proj.

Sharding: core = 4*b + g (b = batch element, g = query-chunk of 512 rows).
No collectives: every core receives the FULL x^T of its batch (bf16,
host-transposed, token-rotated so its own 512 query rows sit first) and
replicates the K^T / V projections for all 2048 keys locally — on this part
the 55us+ fixed cost of a 4-core ring AllGather loses to ~60us of extra
bf16 matmuls that pipeline perfectly.

Schedule (single PE stream, everything else slotted around it):
  Q proj -> K proj ct 0-1 -> attention j=0 with the 16 V-projection steps
  interleaved chunk-by-chunk -> attention j=1..4 with K proj ct 2-5 spread
  as PE filler -> attention j=5 -> output projection.  Each head pair's
  finalize (den -> ones-broadcast matmul -> reciprocal_approx_fast ->
  multiply) is deferred into the next pair's chunk loop.  PSUM: S tiles
  3-deep (6 banks) + one PV accumulator pair (2 banks).

Layouts: all activations transposed ([cols, tokens]) except V (natural),
everything bf16 on the wire and in SBUF; psum accumulation f32.  V carries
a per-head ones column so P@V also yields the softmax denominator; the V
bias is folded into the output bias on the host (sum(P)=1).
"""

import numpy as np

import concourse.bass as bass
import concourse.tile as tile
from concourse import bacc, mybir
from concourse.bass import ts, ds
from concourse.bass_utils import run_bass_kernel_spmd

FP = mybir.dt.float32
FR = mybir.dt.float32r
BF = mybir.dt.bfloat16

P = 128
T = 512            # query rows per core
D = 768            # model dim
H = 12             # heads
DH = 64            # head dim
VA = H * (DH + 1)  # 780: v columns + per-head ones column
KEYS = 2048
DC = D // P        # 6 chunks of the contraction dim
NKC = KEYS // P    # 16 key chunks of 128
NKT = KEYS // T    # 4 key chunks of 512
SCALE = DH ** -0.5


def build_nc():
    nc = bacc.Bacc(
        "TRN2",
        target_bir_lowering=False,
        debug=False,
        enable_asserts=False,
        num_devices=8,
    )
    import os
    dbg = {}
    for name, shape in (
        ("dQT", [P, DC, T]), ("dKT", [P, DC, KEYS]),
        ("dV", [P, NKC, VA]), ("dOT", [P, DC, T]),
    ):
        if name[1:] in os.environ.get("KDBG", "").split(","):
            dbg[name[1:]] = nc.dram_tensor(name, shape, BF, kind="ExternalOutput").ap()

    xT = nc.dram_tensor("xT", [D, KEYS], BF, kind="ExternalInput").ap()
    wq = nc.dram_tensor("wq", [DC, P, DC, P], BF, kind="ExternalInput").ap()
    wk = nc.dram_tensor("wk", [DC, P, DC, P], BF, kind="ExternalInput").ap()
    wv = nc.dram_tensor("wv", [D, VA], BF, kind="ExternalInput").ap()
    bq = nc.dram_tensor("bq", [D], FP, kind="ExternalInput").ap()
    bk = nc.dram_tensor("bk", [D], FP, kind="ExternalInput").ap()
    wo = nc.dram_tensor("wo", [D, D], BF, kind="ExternalInput").ap()
    bo = nc.dram_tensor("bo", [D], FP, kind="ExternalInput").ap()
    out = nc.dram_tensor("out", [T, D], FP, kind="ExternalOutput").ap()

    with tile.TileContext(nc) as tc:
        _build_body(tc, xT, wq, wk, wv, bq, bk, wo, bo, out, dbg)
    nc.compile()
    return nc


def _build_body(tc, xT_d, wq, wk, wv, bq, bk, wo, bo, out, dbg=None):
    nc = tc.nc
    Add = mybir.AluOpType.add
    Mult = mybir.AluOpType.mult
    Exp = mybir.ActivationFunctionType.Exp

    big = tc.alloc_tile_pool(name="big", bufs=1)
    stream = tc.alloc_tile_pool(name="stream", bufs=2)
    singles = tc.alloc_tile_pool(name="singles", bufs=1)
    psum = tc.alloc_tile_pool(name="psum", bufs=2, space="PSUM")

    # b2: [128, 1024] f32 = 2 psum banks; bufs=3 -> 6 banks.
    def b2(name):
        return psum.tile([P, 2 * T], FP, tag="b2", bufs=3, name=name)

    # pv: attention accumulator, 2 banks, single-buffered.
    def bpv(name):
        return psum.tile([P, 2 * T], FP, tag="pv", bufs=1, name=name)

    # ---- persistent SBUF tensors ----
    xT = big.tile([P, DC, KEYS], BF)     # x^T, all tokens (rotated)
    QT = big.tile([P, DC, T], BF)        # Q^T for own 512 rows (biased)
    KT = big.tile([P, DC, KEYS], BF)     # K^T all keys (biased)
    V = big.tile([P, NKC, VA], BF)       # V all keys (+ones cols)
    OT = big.tile([P, DC, T], BF)        # attention output, transposed
    wv_sb = big.tile([P, DC, VA], BF)
    wo_sb = big.tile([P, DC, D], BF)

    # ---- constants ----
    ones_bf = singles.tile([1, DH], BF)
    nc.vector.memset(ones_bf, 1.0)
    bq_sb = singles.tile([P, DC], FP)
    bk_sb = singles.tile([P, DC], FP)
    bo_bc = singles.tile([P, D], FP)

    # ---- input DMAs: prioritize what phase 1 (Q proj) needs ----
    wq_sb = big.tile([P, DC, DC, P], BF)   # [p, ct, o, c]
    wk_sb = big.tile([P, DC, DC, P], BF)
    for dc in range(DC):
        nc.sync.dma_start(xT[:, dc, 0:T], xT_d[ts(dc, P), 0:T])
    for ct in range(DC):
        nc.sync.dma_start(wq_sb[:, ct, :, :], wq[ct])
    nc.sync.dma_start(bq_sb, bq.rearrange("(o p) -> p o", p=P))
    nc.sync.dma_start(bk_sb, bk.rearrange("(o p) -> p o", p=P))
    for ct in range(DC):
        nc.sync.dma_start(wk_sb[:, ct, :, :], wk[ct])
    for dc in range(DC):
        nc.sync.dma_start(xT[:, dc, T:KEYS], xT_d[ts(dc, P), T:KEYS])
    for dc in range(DC):
        nc.sync.dma_start(wv_sb[:, dc, :], wv[ts(dc, P), :])
    for dc in range(DC):
        nc.sync.dma_start(wo_sb[:, dc, :], wo[ts(dc, P), :])
    nc.gpsimd.dma_start(
        out=bo_bc, in_=bass.AP(tensor=bo.tensor, offset=bo.offset, ap=[[0, P], *bo.ap])
    )

    # ---- phase 1: Q^T projection (own 512 rows) ----
    for ct in range(DC):
        pq = b2("pq")
        for dc in range(DC):
            nc.tensor.matmul(
                pq[:, :T], wq_sb[:, ct, dc, :], xT[:, dc, 0:T],
                start=(dc == 0), stop=(dc == DC - 1),
            )
        nc.scalar.add(QT[:, ct, :], pq[:, :T], bq_sb[:, ct : ct + 1])

    # ---- phase 2: K^T projection; ct 0-1 upfront, ct 2-5 interleaved into
    # the attention loop as PE filler work.
    def k_group(ct, kc):
        pk = b2("pk")
        for dc in range(DC):
            nc.tensor.matmul(
                pk[:, :T], wk_sb[:, ct, dc, :], xT[:, dc, ts(kc, T)],
                start=(dc == 0), stop=(dc == DC - 1),
            )
        nc.scalar.add(KT[:, ct, ts(kc, T)], pk[:, :T], bk_sb[:, ct : ct + 1])

    for kc in range(NKT):
        k_group(0, kc)

    # ---- phase 3+4: V projection (all keys) interleaved with attention j=0
    # V tile tt covers key chunk c=tt (128 tokens); attention consumes chunks
    # in the same order, so j=0 can run inside the V loop.
    def v_step(tt):
        pv = b2("pvproj")
        for dc in range(DC):
            for lo, sz in ((0, T), (T, VA - T)):
                nc.tensor.matmul(
                    pv[:, ds(lo, sz)],
                    xT[:, dc, ts(tt, P)],
                    wv_sb[:, dc, ds(lo, sz)],
                    start=(dc == 0), stop=(dc == DC - 1),
                )
        nc.vector.tensor_copy(out=V[:, tt, :], in_=pv[:, :VA])
        ones_ap = V[:, tt, :].rearrange("p (h d1) -> p h d1", d1=DH + 1)[:, :, DH]
        nc.vector.memset(ones_ap, 1.0)

    def attn_j(j, interleave_v=False, fill_k=(), fin_prev=None):
        """Attention for head pair (2j, 2j+1) over all 16 key chunks.
        Returns a finalize closure (run it one j later to pipeline).
        If interleave_v, the V-projection steps are interleaved; fill_k
        closures are spread across the chunk loop as PE filler work."""
        fill_k = list(fill_k)
        pv_acc = None  # allocated lazily at the first PV accumulation
        ps_tiles = {}

        def s_step(c):
            ps = b2(f"ps{j}_{c}")
            ps_tiles[c] = ps
            for hl, off in ((0, 0), (1, DH)):
                nc.tensor.matmul(
                    ps[:, ds(hl * T, T)],
                    KT[ds(off, DH), j, ts(c, P)],
                    QT[ds(off, DH), j, :],
                    start=True, stop=True,
                )

        for c0 in range(2):
            if interleave_v:
                v_step(c0)
            s_step(c0)
        for c in range(NKC):
            es = stream.tile([P, 2 * T], BF, tag="expS", bufs=4, name="es")
            nc.scalar.activation(es, ps_tiles[c][:, :], Exp, scale=SCALE)
            if c == 0 and fin_prev is not None:
                fin_prev()
            if c + 2 < NKC:
                s_step(c + 2)
                if interleave_v:
                    v_step(c + 2)
            if fill_k and c % 4 == 1:
                fill_k.pop(0)()
            if pv_acc is None:
                pv_acc = bpv(f"pv{j}")  # h0: [:65, :512], h1: [:65, 512:]
            for hl in (0, 1):
                nc.tensor.matmul(
                    pv_acc[: DH + 1, ds(hl * T, T)],
                    V[:, c, ds((2 * j + hl) * (DH + 1), DH + 1)],
                    es[:, ds(hl * T, T)],
                    start=(c == 0), stop=(c == NKC - 1),
                )

        def finalize():
            den_bf = stream.tile([1, 2 * T], BF, tag="den", bufs=2, name="den_bf")
            nc.vector.tensor_copy(out=den_bf, in_=pv_acc[DH : DH + 1, :])
            bc = b2(f"bc{j}")
            for hl in (0, 1):
                nc.tensor.matmul(
                    bc[:DH, ds(hl * T, T)], ones_bf, den_bf[:, ds(hl * T, T)],
                    start=True, stop=True,
                )
            bc_sb = stream.tile([DH, 2 * T], FP, tag="bcs", bufs=2, name="bc_sb")
            nc.vector.tensor_copy(out=bc_sb, in_=bc[:DH, :])
            recip = stream.tile([DH, 2 * T], FP, tag="recip", bufs=2, name="recip")
            nc.vector.reciprocal_approx_fast(out=recip, in_=bc_sb)
            for hl in (0, 1):
                nc.vector.tensor_tensor(
                    out=OT[ds(hl * DH, DH), j, :],
                    in0=pv_acc[:DH, ds(hl * T, T)],
                    in1=recip[:, ds(hl * T, T)], op=Mult,
                )

        return finalize

    fin = None
    for j in range(DC):
        fill_k = ()
        if j <= 4:
            ct = j + 1
            fill_k = tuple(
                (lambda ct=ct, kc=kc: k_group(ct, kc)) for kc in range(NKT)
            )
        fin = attn_j(j, interleave_v=(j == 0), fill_k=fill_k, fin_prev=fin)
    fin()

    # ---- phase 6: output projection ----
    for tt in range(T // P):
        po = b2("po")
        for dc in range(DC):
            for lo, sz in ((0, T), (T, D - T)):
                nc.tensor.matmul(
                    po[:, ds(lo, sz)],
                    OT[:, dc, ts(tt, P)],
                    wo_sb[:, dc, ds(lo, sz)],
                    start=(dc == 0), stop=(dc == DC - 1),
                )
        o_stage = stream.tile([P, D], FP, tag="ost", bufs=2, name="o_stage")
        nc.vector.tensor_tensor(out=o_stage, in0=po[:, :D], in1=bo_bc, op=Add)
        nc.sync.dma_start(out[ts(tt, P), :], o_stage)

    if dbg:
        tiles = {"QT": QT, "KT": KT, "V": V, "OT": OT}
        for name, dap in dbg.items():
            nc.sync.dma_start(dap, tiles[name])

    for pool in (psum, singles, stream, big):
        pool.release()


_CACHE = {}


def _get_nc():
    if "nc" not in _CACHE:
        _CACHE["nc"] = build_nc()
    return _CACHE["nc"]


def _prep_inputs(x, w_qkv, b_qkv, w_out, b_out):
    import ml_dtypes

    bf16 = ml_dtypes.bfloat16
    x = np.asarray(x, np.float32)
    w_qkv = np.asarray(w_qkv, np.float32)
    b_qkv = np.asarray(b_qkv, np.float32)
    w_out = np.asarray(w_out, np.float32)
    b_out = np.asarray(b_out, np.float32)

    wq_n = w_qkv[:, 0:768]
    wk_n = w_qkv[:, 768:1536]
    wv_raw = w_qkv[:, 1536:2304]
    bq = np.ascontiguousarray(b_qkv[0:768])
    bk = np.ascontiguousarray(b_qkv[768:1536])
    bv_raw = b_qkv[1536:2304]

    # [ct, p, o, c] layout so the per-ct stationary DMA is contiguous
    def w_re(w):
        return np.ascontiguousarray(
            w.reshape(DC, P, DC, P).transpose(2, 1, 0, 3).astype(bf16)
        )

    wq_r = w_re(wq_n)
    wk_r = w_re(wk_n)

    wv = np.zeros((D, VA), np.float32)
    for h in range(H):
        wv[:, h * 65 : h * 65 + 64] = wv_raw[:, h * 64 : (h + 1) * 64]
    wv = wv.astype(bf16)
    # V bias folds into the output bias: softmax rows sum to 1.
    bo_eff = (b_out + bv_raw @ w_out).astype(np.float32)
    wo = np.ascontiguousarray(w_out.astype(bf16))

    in_maps = []
    for b in range(2):
        xb = x[b]
        for g in range(4):
            xrot = np.roll(xb, -g * T, axis=0)
            xTb = np.ascontiguousarray(xrot.T.astype(bf16))
            in_maps.append(
                dict(
                    xT=xTb, wq=wq_r, wk=wk_r, wv=wv, bq=bq, bk=bk,
                    wo=wo, bo=bo_eff,
                )
            )
    return in_maps


def run_on_hw(x, w_qkv, b_qkv, w_out, b_out, **kwargs):
    in_maps = _prep_inputs(x, w_qkv, b_qkv, w_out, b_out)
    res = run_bass_kernel_spmd(_get_nc(), in_maps, core_ids=list(range(8)), **kwargs)
    full = np.empty((2, 2048, D), np.float32)
    for b in range(2):
        for g in range(4):
            full[b, g * T : (g + 1) * T] = res.results[b * 4 + g]["out"]
    return full, res


def kernel(x, w_qkv, b_qkv, w_out, b_out):
    full, _ = run_on_hw(x, w_qkv, b_qkv, w_out, b_out)
    return full
